# revision 1
# baseline (speedup 1.0000x reference)
"""Trainium2 Bass kernel for nn_ConsistencyLoss (N=4096, D=8192, 8 NeuronCores).

loss = sum_{i<j} (log(rowsum_i - E_ij) - logits_ij) * (j - i)
  S = cos-sim Gram matrix of `slots`, logits = S/T, E = exp(logits),
  rowsum_i = sum_k E_ik.

Strategy (matches the row-sharded hint):
  * core c owns rows [512c, 512c+512)
  * device: normalize rows, PE-transpose own shard -> DRAM chunk,
    AllGather chunks -> full transposed slots, then a K=8192 f32r matmul
    producing the [512, 4096] block of S in 4 PSUM accumulators per
    512-wide column block.
  * per output tile: E = Exp(invT * S) on ACT (fused row-sum accum_out);
    W = relu(j - i) on DVE; tensor_tensor_reduce chains accumulate
    sum(E*W), sum(E^2*W), sum(E^3*W), sum(S*W) and the diagonal E_ii
    candidate (identity-mask reduce).
  * host (float64): log(rs - E) = ln(rs) - E/rs - E^2/(2 rs^2) - E^3/(3 rs^3)
    (E/rs <= ~1e-4 for cosine Gram data: exact to fp32 precision), with an
    exact diagonal correction of rowsum, then
    loss = sum_i [ln(rs_i)*SW_i - SEW_i/rs_i - ...] - invT * sum(S*W).
"""

import os
import sys

# Sanitize before any jax import: the device path needs the axon platform.
if os.environ.get("JAX_PLATFORMS", "") in ("cpu", "CPU"):
    del os.environ["JAX_PLATFORMS"]
os.environ.setdefault("MYCRO_LOCAL_CACHE", "1")

if "/opt/trn_rl_repo" not in sys.path:
    sys.path.insert(0, "/opt/trn_rl_repo")

import numpy as np

N, D = 4096, 8192
NC = 8
R = N // NC          # 512 rows per core
P = 128
MT = R // P          # 4 m-tiles per core
KT = D // P          # 64 k-tiles
CB = 512             # column block width
NB = N // CB         # 8 column blocks
EPS = 1e-6

GEMM_DT = os.environ.get("CONSISTENCY_GEMM_DT", "bf16")  # "f32r" | "bf16"

_BUILT = {}


def _build(invT: float, gemm_dt: str, collective: bool = True):
    import concourse.bass as bass  # noqa: F401
    from concourse import bacc
    import concourse.mybir as mybir
    import concourse.tile as tile
    from concourse.masks import make_identity

    dt = mybir.dt
    store_dt = dt.float32r if gemm_dt == "f32r" else dt.bfloat16

    nc = bacc.Bacc("TRN2", target_bir_lowering=False, debug=False, num_devices=NC)

    shard_in = nc.dram_tensor("shard", [R, D], dt.float32, kind="ExternalInput")
    ridx_in = nc.dram_tensor("ridx", [P, MT], dt.float32, kind="ExternalInput")
    jcol_in = nc.dram_tensor("jcol", [P, N], dt.float32, kind="ExternalInput")

    rs_o = nc.dram_tensor("rs", [P, MT * NB], dt.float32, kind="ExternalOutput")
    sew_o = nc.dram_tensor("sew", [P, MT * NB], dt.float32, kind="ExternalOutput")
    se2w_o = nc.dram_tensor("se2w", [P, MT * NB], dt.float32, kind="ExternalOutput")
    se3w_o = nc.dram_tensor("se3w", [P, MT * NB], dt.float32, kind="ExternalOutput")
    b_o = nc.dram_tensor("b", [P, MT * NB], dt.float32, kind="ExternalOutput")
    eii_o = nc.dram_tensor("eii", [P, MT * NB], dt.float32, kind="ExternalOutput")
    ss_o = nc.dram_tensor("ss", [P, MT], dt.float32, kind="ExternalOutput")


    with tile.TileContext(nc) as tc:
        with (
            tc.tile_pool(name="const", bufs=1) as const,
            tc.tile_pool(name="lhsT", bufs=1) as lhsp,
            tc.tile_pool(name="dram", bufs=1, space="DRAM") as dram,
        ):
            ident = const.tile([P, P], dt.float32)
            make_identity(nc, ident[:])

            ridx = const.tile([P, MT], dt.float32)
            nc.sync.dma_start(ridx[:], ridx_in[:])

            # accumulator slots (written per (m, nb) tile, DMA'd out at end)
            rs_sb = const.tile([P, MT * NB], dt.float32)
            sew_sb = const.tile([P, MT * NB], dt.float32)
            se2w_sb = const.tile([P, MT * NB], dt.float32)
            se3w_sb = const.tile([P, MT * NB], dt.float32)
            b_sb = const.tile([P, MT * NB], dt.float32)
            eii_sb = const.tile([P, MT * NB], dt.float32)
            ss_sb = const.tile([P, MT], dt.float32)
            rn_sb = const.tile([P, MT], dt.float32)

            # resident transposed normalized shard: [P(d), KT, MT, P(rows)]
            lhsT = lhsp.tile([P, KT, MT, P], store_dt)

            chunk = dram.tile([KT, P, R], store_dt)
            gathered = dram.tile([NC, KT, P, R], store_dt, addr_space="Shared")

            # ---------------- Phase A: transpose raw shard; fold row-norms
            # into the PSUM->lhsT copy as a multiply with a PE-broadcast tile.
            ones_sb = const.tile([P, P], dt.float32)
            nc.vector.memset(ones_sb[:], 1.0)
            sh_bufs = 2 if gemm_dt == "bf16" else 1
            with (
                tc.tile_pool(name="pash", bufs=sh_bufs) as pash,
                tc.tile_pool(name="pa1", bufs=2) as pa1,
                tc.tile_pool(name="paps", bufs=4, space="PSUM") as paps,
                tc.tile_pool(name="pabs", bufs=2, space="PSUM") as pabs,
            ):
                NS = 4            # strips per m-tile
                SW_ = D // NS     # 2048 strip width
                KS = SW_ // P     # 16 k-tiles per strip
                for m in range(MT):
                    sh = pash.tile([P, D], dt.float32, tag="sh")
                    nc.sync.dma_start(sh[:], shard_in[m * P:(m + 1) * P, :])
                    ssp = pa1.tile([P, NS], dt.float32, tag="ssp")
                    sq = pa1.tile([P, SW_], dt.float32, tag="sq")
                    for s in range(NS):
                        sl = sh[:, s * SW_:(s + 1) * SW_]
                        nc.scalar.activation(
                            sq[:], sl, mybir.ActivationFunctionType.Square,
                            accum_out=ssp[:, s:s + 1],
                        )
                    nc.vector.reduce_sum(
                        ss_sb[:, m:m + 1], ssp[:], axis=mybir.AxisListType.X
                    )
                    nrm = pa1.tile([P, 1], dt.float32, tag="nrm")
                    nc.scalar.activation(
                        nrm[:], ss_sb[:, m:m + 1], mybir.ActivationFunctionType.Sqrt
                    )
                    nc.vector.tensor_scalar_max(nrm[:], nrm[:], EPS)
                    nc.vector.reciprocal(rn_sb[:, m:m + 1], nrm[:])
                    # rn broadcast tile: rnb[p, r] = rn[m*128 + r] for all p
                    ptr1 = pabs.tile([P, P], dt.float32, tag="ptr1")
                    nc.tensor.transpose(
                        ptr1[:1, :], rn_sb[:, m:m + 1], ident[:]
                    )
                    rnrow = pa1.tile([P, P], dt.float32, tag="rnrow")
                    nc.vector.tensor_copy(rnrow[:1, :], ptr1[:1, :])
                    ptr2 = pabs.tile([P, P], dt.float32, tag="ptr2")
                    nc.tensor.matmul(
                        ptr2[:], ones_sb[:1, :], rnrow[:1, :],
                        start=True, stop=True,
                    )
                    rnb = pa1.tile([P, P], dt.float32, tag="rnb")
                    nc.vector.tensor_copy(rnb[:], ptr2[:])
                    for s in range(NS):
                        for kk in range(KS):
                            k = s * KS + kk
                            pst = paps.tile([P, P], dt.float32, tag="pst")
                            nc.tensor.transpose(
                                pst[:], sh[:, k * P:(k + 1) * P], ident[:]
                            )
                            nc.vector.tensor_tensor(
                                lhsT[:, k, m, :], pst[:], rnb[:],
                                mybir.AluOpType.mult,
                            )
                        nc.sync.dma_start(
                            chunk[s * KS:(s + 1) * KS, :, m * P:(m + 1) * P],
                            lhsT[:, s * KS:(s + 1) * KS, m, :],
                        )

            # ---------------- Phase B: AllGather ---------------------------
            if collective:
                nc.gpsimd.collective_compute(
                    "AllGather",
                    mybir.AluOpType.bypass,
                    replica_groups=[list(range(NC))],
                    ins=[chunk.opt()],
                    outs=[gathered.opt()],
                )

            # ---------------- Phase C: matmul + fused reductions -----------
            with (
                tc.tile_pool(name="jc", bufs=1) as jcp,
                tc.tile_pool(name="rhs", bufs=4 if gemm_dt == "bf16" else 3) as rhsp,
                tc.tile_pool(name="scr", bufs=2) as scr,
                tc.tile_pool(name="mps", bufs=2, space="PSUM") as mps,
            ):
                jcol = jcp.tile([P, N], dt.float32)
                nc.sync.dma_start(jcol[:], jcol_in[:])

                KQ = 4  # k-tiles per rhs DMA (1 MiB-ish loads)
                for nb in range(NB):
                    psums = [
                        mps.tile([P, CB], dt.float32, tag=f"ps{m}",
                                 name=f"ps_{nb}_{m}")
                        for m in range(MT)
                    ]
                    for kq in range(KT // KQ):
                        rq = rhsp.tile([P, KQ, CB], store_dt, tag="rq")
                        nc.sync.dma_start(
                            rq[:],
                            gathered[nb, kq * KQ:(kq + 1) * KQ].rearrange(
                                "k p n -> p k n"
                            ),
                        )
                        for kk in range(KQ):
                            k = kq * KQ + kk
                            for m in range(MT):
                                nc.tensor.matmul(
                                    psums[m][:],
                                    lhsT[:, k, m, :],
                                    rq[:, kk, :],
                                    start=(k == 0),
                                    stop=(k == KT - 1),
                                )
                    for m in range(MT):
                        idx = m * NB + nb
                        e_t = scr.tile([P, CB], dt.float32, tag="e")
                        nc.scalar.activation(
                            e_t[:], psums[m][:], mybir.ActivationFunctionType.Exp,
                            scale=invT, accum_out=rs_sb[:, idx:idx + 1],
                        )
                        w_t = scr.tile([P, CB], dt.float32, tag="w")
                        nc.vector.tensor_scalar(
                            w_t[:], jcol[:, nb * CB:(nb + 1) * CB],
                            ridx[:, m:m + 1], 0.0,
                            mybir.AluOpType.subtract, mybir.AluOpType.max,
                        )
                        ew_t = scr.tile([P, CB], dt.float32, tag="ew")
                        nc.vector.tensor_tensor(
                            ew_t[:], e_t[:], w_t[:], mybir.AluOpType.mult
                        )
                        nc.vector.reduce_sum(
                            sew_sb[:, idx:idx + 1], ew_t[:],
                            axis=mybir.AxisListType.X,
                        )
                        e2w_t = scr.tile([P, CB], dt.float32, tag="e2w")
                        nc.vector.tensor_tensor(
                            e2w_t[:], ew_t[:], e_t[:], mybir.AluOpType.mult
                        )
                        nc.vector.reduce_sum(
                            se2w_sb[:, idx:idx + 1], e2w_t[:],
                            axis=mybir.AxisListType.X,
                        )
                        e3w_t = scr.tile([P, CB], dt.float32, tag="e3w")
                        nc.vector.tensor_tensor(
                            e3w_t[:], e2w_t[:], e_t[:], mybir.AluOpType.mult
                        )
                        nc.vector.reduce_sum(
                            se3w_sb[:, idx:idx + 1], e3w_t[:],
                            axis=mybir.AxisListType.X,
                        )
                        bw_t = scr.tile([P, CB], dt.float32, tag="bw")
                        nc.vector.tensor_tensor(
                            bw_t[:], psums[m][:], w_t[:], mybir.AluOpType.mult
                        )
                        nc.vector.reduce_sum(
                            b_sb[:, idx:idx + 1], bw_t[:],
                            axis=mybir.AxisListType.X,
                        )
                        de_t = scr.tile([P, P], dt.float32, tag="de")
                        nc.vector.tensor_tensor(
                            de_t[:], e_t[:, m * P:(m + 1) * P], ident[:],
                            mybir.AluOpType.mult,
                        )
                        nc.vector.reduce_sum(
                            eii_sb[:, idx:idx + 1], de_t[:],
                            axis=mybir.AxisListType.X,
                        )

            nc.sync.dma_start(rs_o[:], rs_sb[:])
            nc.sync.dma_start(sew_o[:], sew_sb[:])
            nc.sync.dma_start(se2w_o[:], se2w_sb[:])
            nc.sync.dma_start(se3w_o[:], se3w_sb[:])
            nc.sync.dma_start(b_o[:], b_sb[:])
            nc.sync.dma_start(eii_o[:], eii_sb[:])
            nc.sync.dma_start(ss_o[:], ss_sb[:])

    if not nc.is_finalized():
        nc.finalize()
    return nc


def _run_device(slots: np.ndarray, invT: float, trace: bool = False):
    from concourse.bass_utils import run_bass_kernel_spmd

    key = (GEMM_DT, round(invT, 9))
    if key not in _BUILT:
        _BUILT[key] = _build(invT, GEMM_DT)
    nc = _BUILT[key]

    jcol = np.broadcast_to(
        np.arange(N, dtype=np.float32), (P, N)
    ).copy()
    in_maps = []
    for c in range(NC):
        ridx = (
            c * R
            + P * np.arange(MT, dtype=np.float32)[None, :]
            + np.arange(P, dtype=np.float32)[:, None]
        ).astype(np.float32)
        in_maps.append(
            {
                "shard": np.ascontiguousarray(slots[c * R:(c + 1) * R]),
                "ridx": np.ascontiguousarray(ridx),
                "jcol": jcol,
            }
        )
    res = run_bass_kernel_spmd(
        nc, in_maps, core_ids=list(range(NC)), trace=trace
    )
    return res


def _assemble(outs, invT: float, length: int):
    """Host-side float64 assembly of the loss from per-core partial sums."""
    loss = 0.0
    for c in range(NC):
        o = outs[c]
        rs = o["rs"].astype(np.float64).reshape(P, MT, NB).sum(-1)
        sew = o["sew"].astype(np.float64).reshape(P, MT, NB).sum(-1)
        se2w = o["se2w"].astype(np.float64).reshape(P, MT, NB).sum(-1)
        se3w = o["se3w"].astype(np.float64).reshape(P, MT, NB).sum(-1)
        bsum = o["b"].astype(np.float64).sum()
        eii = o["eii"].astype(np.float64).reshape(P, MT, NB)[:, :, c]
        ss = o["ss"].astype(np.float64)

        # exact diagonal correction: replace measured E_ii (matmul-rounded)
        # with exp(invT * ss/max(sqrt(ss),eps)^2) from the exact row norms
        nrm = np.maximum(np.sqrt(ss), EPS)
        sii = ss / (nrm * nrm)
        rs_corr = rs - eii + np.exp(invT * sii)

        i_idx = (
            c * R
            + P * np.arange(MT, dtype=np.float64)[None, :]
            + np.arange(P, dtype=np.float64)[:, None]
        )
        swt = (N - 1 - i_idx) * (N - i_idx) / 2.0

        A = (
            np.log(rs_corr) * swt
            - sew / rs_corr
            - se2w / (2.0 * rs_corr**2)
            - se3w / (3.0 * rs_corr**3)
        )
        loss += A.sum() - invT * bsum
    norm_loss = loss / (((length - 1) * (length - 1)) / 2.0)
    return np.float32(loss), np.float32(norm_loss)


def _kernel_numpy_fallback(slots, length, temperature):
    """Emergency CPU path (used only if the device run fails)."""
    s = slots.astype(np.float64)
    nrm = np.maximum(np.sqrt((s * s).sum(1)), EPS)
    S = (s @ s.T) / (nrm[:, None] * nrm[None, :])
    logits = S / float(temperature)
    E = np.exp(logits)
    den = E.sum(1)[:, None] - E
    idx = np.arange(int(length))
    pen = (idx[None, :] - idx[:, None]).astype(np.float64)
    per = (np.log(den) - logits) * pen
    loss = per[pen > 0].sum()
    norm_loss = loss / (((length - 1) * (length - 1)) / 2.0)
    return np.float32(loss), np.float32(norm_loss)


def kernel(slots, length, temperature):
    slots = np.ascontiguousarray(np.asarray(slots, dtype=np.float32))
    assert slots.shape == (N, D), slots.shape
    length_i = int(length)
    invT = float(1.0 / np.float32(temperature))
    try:
        res = _run_device(slots, invT)
        return _assemble(res.results, invT, length_i)
    except Exception as e:  # pragma: no cover - emergency path
        sys.stderr.write(f"[kernel] device path FAILED ({e!r})\n")
        if os.environ.get("CONSISTENCY_NO_FALLBACK"):
            raise
        sys.stderr.write("[kernel] using numpy fallback\n")
        return _kernel_numpy_fallback(slots, length_i, temperature)


if __name__ == "__main__":
    x = np.random.default_rng(0).standard_normal((N, D)).astype(np.float32)
    print(kernel(x, N, np.float32(0.1)))

